# revision 7
# baseline (speedup 1.0000x reference)
"""AWQ linear kernel for Trainium2, 8-core column-parallel.

Computes y = x @ (qweight * scales).T + bias with
  x: [4, 4096, 4096] f32, qweight: [16384, 4096] int32 (values in [-15, 15]),
  scales: [16384, 1] f32, bias: [16384] f32.

Sharding: qweight/scales/bias are split along out_features across 8 cores
(column-parallel); x is replicated; each core computes its [M, 2048] output
shard and the host concatenates.

Math strategy: the integer qweight values are exactly representable in bf16,
so the matmul runs in bf16 against the *raw* integer weights and the
per-output-channel scale is applied to the fp32 PSUM result afterwards
(mathematically identical to dequantize-then-matmul). The only quantization
error is the bf16 rounding of x. An optional SPLIT mode represents
x = hi + lo with two bf16 arrays and accumulates both matmuls into the same
PSUM for near-fp32 accuracy at 2x PE cost.

Device-side data flow (per core):
  phase A: x f32 [M, K] -> bf16 [M, K] DRAM scratch (gpsimd cast-DMA, or
           DVE cast when SPLIT needs the hi/lo subtraction)
  phase B: XBAR DMA-transpose loads of x_bf16 -> SBUF [k, m] tiles;
           weights (host-pretransposed [K, Nc] bf16) resident in SBUF;
           PE matmuls accumulate over K into PSUM; DVE applies
           scale/bias on eviction; result DMA'd to DRAM.
"""

import os
from contextlib import ExitStack

import numpy as np
import ml_dtypes

import concourse.bass as bass
import concourse.tile as tile
from concourse import bacc, mybir
from concourse.bass_utils import run_bass_kernel_spmd

P = 128

# Full-problem constants
B, S, DIN, DOUT = 4, 4096, 4096, 16384
M_FULL = B * S          # 16384 rows of x
K_FULL = DIN            # 4096 contraction
N_CORES = 8
N_CORE_FULL = DOUT // N_CORES  # 2048 output features per core

# Tunables
M_CHUNK = int(os.environ.get("AWQ_M_CHUNK", "256"))   # x rows per compute chunk
N_SPLIT = int(os.environ.get("AWQ_N_SPLIT", "1"))     # weight residency chunks
SPLIT = os.environ.get("AWQ_SPLIT", "0") == "1"       # hi/lo x split (accuracy)
N_TILE = 512                                          # matmul moving free dim
A_CHUNK = 2048                                        # phase-A DVE chunk cols


def build_module(M, K, N_core, m_chunk, n_split, split, enable_asserts=False):
    """Emit the full tile program for one core (same program for all cores)."""
    KS = K // P
    assert M % m_chunk == 0 and m_chunk % P == 0
    assert N_core % n_split == 0
    n_chunk = N_core // n_split
    assert n_chunk % N_TILE == 0
    nt_per = n_chunk // N_TILE
    msb_per = m_chunk // P
    f32 = mybir.dt.float32
    bf16 = mybir.dt.bfloat16

    nc = bacc.Bacc(
        "TRN2",
        target_bir_lowering=False,
        debug=False,
        enable_asserts=enable_asserts,
        num_devices=N_CORES,
    )

    x_ap = nc.dram_tensor("x", [M, K], f32, kind="ExternalInput").ap()
    wt_ap = nc.dram_tensor("wt", [K, N_core], bf16, kind="ExternalInput").ap()
    sc_ap = nc.dram_tensor("sc", [1, N_core], f32, kind="ExternalInput").ap()
    bi_ap = nc.dram_tensor("bi", [1, N_core], f32, kind="ExternalInput").ap()
    out_ap = nc.dram_tensor("out", [M, N_core], f32, kind="ExternalOutput").ap()

    n_mchunks = M // m_chunk
    terms = 2 if split else 1

    with tile.TileContext(nc) as tc, ExitStack() as ctx:
        dram = ctx.enter_context(
            tc.tile_pool(name="dram", bufs=n_mchunks * terms, space="DRAM")
        )
        consts = ctx.enter_context(tc.tile_pool(name="consts", bufs=1))
        wt_pool = ctx.enter_context(tc.tile_pool(name="wt_pool", bufs=1))
        xt_pool = ctx.enter_context(tc.tile_pool(name="xt_pool", bufs=2))
        ev_pool = ctx.enter_context(tc.tile_pool(name="ev_pool", bufs=2))
        psum = ctx.enter_context(tc.tile_pool(name="psum", bufs=8, space="PSUM"))
        # XBAR transpose instructions block their issuing HWDGE engine for the
        # whole transfer; alternate between the two HWDGE engines (SP + ACT).
        hwdge = [nc.sync, nc.scalar]

        # Broadcast scale/bias across partitions once.
        sc_sb = consts.tile([P, N_core], f32, name="sc_sb")
        nc.scalar.dma_start(sc_sb[:], sc_ap.to_broadcast((P, N_core)))
        bi_sb = consts.tile([P, N_core], f32, name="bi_sb")
        nc.scalar.dma_start(bi_sb[:], bi_ap.to_broadcast((P, N_core)))

        # ---- Phase A: x f32 -> bf16 (and lo term when split) in DRAM ----
        xb_tiles = []  # per m-chunk, list of per-term DRAM tiles
        if not split:
            for mc in range(n_mchunks):
                xb = dram.tile([m_chunk, K], bf16, name=f"xb_{mc}", tag="xb")
                nc.gpsimd.dma_start(
                    out=xb[:], in_=x_ap[mc * m_chunk : (mc + 1) * m_chunk, :]
                )
                xb_tiles.append([xb])
        else:
            a_pool = ctx.enter_context(tc.tile_pool(name="a_pool", bufs=3))
            ah_pool = ctx.enter_context(tc.tile_pool(name="ah_pool", bufs=3))
            al_pool = ctx.enter_context(tc.tile_pool(name="al_pool", bufs=3))
            a_chunk = min(A_CHUNK, K)
            for mc in range(n_mchunks):
                xbh = dram.tile([m_chunk, K], bf16, name=f"xbh_{mc}", tag="xbh")
                xbl = dram.tile([m_chunk, K], bf16, name=f"xbl_{mc}", tag="xbl")
                for sub in range(m_chunk // P):
                    r0 = mc * m_chunk + sub * P
                    for kc in range(K // a_chunk):
                        c0 = kc * a_chunk
                        a_in = a_pool.tile([P, a_chunk], f32, name="a_in")
                        nc.sync.dma_start(
                            a_in[:], x_ap[r0 : r0 + P, c0 : c0 + a_chunk]
                        )
                        a_hi = ah_pool.tile([P, a_chunk], bf16, name="a_hi")
                        nc.vector.tensor_copy(a_hi[:], a_in[:])
                        nc.sync.dma_start(
                            xbh[sub * P : sub * P + P, c0 : c0 + a_chunk], a_hi[:]
                        )
                        a_lo = al_pool.tile([P, a_chunk], bf16, name="a_lo")
                        nc.vector.tensor_sub(a_lo[:], a_in[:], a_hi[:])
                        nc.sync.dma_start(
                            xbl[sub * P : sub * P + P, c0 : c0 + a_chunk], a_lo[:]
                        )
                xb_tiles.append([xbh, xbl])

        # ---- Phase B: matmul sweep ----
        wt_re = wt_ap.rearrange("(ks p) n -> p ks n", p=P)
        for ncn in range(n_split):
            wt_sb = wt_pool.tile([P, KS, n_chunk], bf16, name=f"wt_{ncn}", tag="wt")
            nc.scalar.dma_start(
                wt_sb[:], wt_re[:, :, ncn * n_chunk : (ncn + 1) * n_chunk]
            )
            for mc in range(n_mchunks):
                ps = [
                    [
                        psum.tile([P, N_TILE], f32, name=f"ps_{msb}_{nt}", tag="ps")
                        for nt in range(nt_per)
                    ]
                    for msb in range(msb_per)
                ]
                for ti in range(terms):
                    xb = xb_tiles[mc][ti]
                    xt = xt_pool.tile([P, KS, m_chunk], bf16, name="xt", tag="xt")
                    for ks in range(KS):
                        nc.sync.dma_start_transpose(
                            xt[:, ks, :], xb[:, ks * P : (ks + 1) * P]
                        )
                    for ks in range(KS):
                        for msb in range(msb_per):
                            lhsT = xt[:, ks, msb * P : (msb + 1) * P]
                            for nt in range(nt_per):
                                nc.tensor.matmul(
                                    ps[msb][nt][:],
                                    lhsT,
                                    wt_sb[:, ks, nt * N_TILE : (nt + 1) * N_TILE],
                                    start=(ti == 0 and ks == 0),
                                    stop=(ti == terms - 1 and ks == KS - 1),
                                )
                for msb in range(msb_per):
                    r0 = mc * m_chunk + msb * P
                    for nt in range(nt_per):
                        c0 = ncn * n_chunk + nt * N_TILE
                        ev = ev_pool.tile([P, N_TILE], f32, name="ev", tag="ev")
                        nc.vector.tensor_mul(
                            ev[:], ps[msb][nt][:], sc_sb[:, c0 : c0 + N_TILE]
                        )
                        nc.vector.tensor_add(
                            ev[:], ev[:], bi_sb[:, c0 : c0 + N_TILE]
                        )
                        nc.scalar.dma_start(
                            out_ap[r0 : r0 + P, c0 : c0 + N_TILE], ev[:]
                        )

    nc.compile()
    return nc


_BUILT = {}


def _get_module():
    key = (M_FULL, K_FULL, N_CORE_FULL, M_CHUNK, N_SPLIT, SPLIT)
    if key not in _BUILT:
        _BUILT[key] = build_module(
            M_FULL, K_FULL, N_CORE_FULL, M_CHUNK, N_SPLIT, SPLIT
        )
    return _BUILT[key]


def kernel(x, qweight, scales, bias):
    bf = ml_dtypes.bfloat16
    x2d = np.ascontiguousarray(x.reshape(M_FULL, K_FULL).astype(np.float32, copy=False))
    scales = np.asarray(scales, dtype=np.float32).reshape(DOUT)
    bias = np.asarray(bias, dtype=np.float32).reshape(DOUT)

    in_maps = []
    for c in range(N_CORES):
        lo, hi = c * N_CORE_FULL, (c + 1) * N_CORE_FULL
        # Weight repack: transpose to [K, Nc]; int values <= 15 are exact in bf16.
        wt_c = np.ascontiguousarray(qweight[lo:hi, :].T).astype(bf)
        in_maps.append(
            {
                "x": x2d,
                "wt": wt_c,
                "sc": scales[lo:hi].reshape(1, N_CORE_FULL),
                "bi": bias[lo:hi].reshape(1, N_CORE_FULL),
            }
        )

    nc = _get_module()
    trace = os.environ.get("AWQ_TRACE", "0") == "1"
    res = run_bass_kernel_spmd(
        nc, in_maps, core_ids=list(range(N_CORES)), trace=trace
    )
    if trace:
        kernel.last_exec_time_ns = res.exec_time_ns
        kernel.last_results = res

    out = np.empty((M_FULL, DOUT), dtype=np.float32)
    for c in range(N_CORES):
        out[:, c * N_CORE_FULL : (c + 1) * N_CORE_FULL] = res.results[c]["out"]
    return out.reshape(B, S, DOUT)


# revision 10
# speedup vs baseline: 1.3055x; 1.3055x over previous
"""AWQ linear kernel for Trainium2, 8-core column-parallel.

Computes y = x @ (qweight * scales).T + bias with
  x: [4, 4096, 4096] f32, qweight: [16384, 4096] int32 (values in [-15, 15]),
  scales: [16384, 1] f32, bias: [16384] f32.

Sharding: qweight/scales/bias are split along out_features across 8 cores
(column-parallel); x is replicated; each core computes its [M, 2048] output
shard and the host concatenates.

Math strategy: the integer qweight values are exactly representable in bf16,
so the matmul runs in bf16 against the *raw* integer weights and the
per-output-channel scale is applied to the fp32 PSUM result afterwards
(mathematically identical to dequantize-then-matmul). The only quantization
error is the bf16 rounding of x. An optional SPLIT mode represents
x = hi + lo with two bf16 arrays and accumulates both matmuls into the same
PSUM for near-fp32 accuracy at 2x PE cost.

Device-side data flow (per core):
  phase A: x f32 [M, K] -> bf16 [M, K] DRAM scratch (gpsimd cast-DMA, or
           DVE cast when SPLIT needs the hi/lo subtraction)
  phase B: XBAR DMA-transpose loads of x_bf16 -> SBUF [k, m] tiles;
           weights (host-pretransposed [K, Nc] bf16) resident in SBUF;
           PE matmuls accumulate over K into PSUM; DVE applies
           scale/bias on eviction; result DMA'd to DRAM.
"""

import os
from contextlib import ExitStack

import numpy as np
import ml_dtypes

import concourse.bass as bass
import concourse.tile as tile
from concourse import bacc, mybir
from concourse.bass_utils import run_bass_kernel_spmd

P = 128

# Full-problem constants
B, S, DIN, DOUT = 4, 4096, 4096, 16384
M_FULL = B * S          # 16384 rows of x
K_FULL = DIN            # 4096 contraction
N_CORES = 8
N_CORE_FULL = DOUT // N_CORES  # 2048 output features per core

# Tunables
M_CHUNK = int(os.environ.get("AWQ_M_CHUNK", "256"))   # x rows per compute chunk
N_SPLIT = int(os.environ.get("AWQ_N_SPLIT", "1"))     # weight residency chunks
SPLIT = os.environ.get("AWQ_SPLIT", "0") == "1"       # hi/lo x split (accuracy)
N_TILE = 512                                          # matmul moving free dim
A_CHUNK = 2048                                        # phase-A DVE chunk cols
XT_G = int(os.environ.get("AWQ_XT_G", "8"))           # ks per xt sub-tile group


def build_module(M, K, N_core, m_chunk, n_split, split, enable_asserts=False):
    """Emit the full tile program for one core (same program for all cores)."""
    KS = K // P
    assert M % m_chunk == 0 and m_chunk % P == 0
    assert N_core % n_split == 0
    n_chunk = N_core // n_split
    assert n_chunk % N_TILE == 0
    nt_per = n_chunk // N_TILE
    msb_per = m_chunk // P
    f32 = mybir.dt.float32
    bf16 = mybir.dt.bfloat16

    nc = bacc.Bacc(
        "TRN2",
        target_bir_lowering=False,
        debug=False,
        enable_asserts=enable_asserts,
        num_devices=N_CORES,
    )

    x_ap = nc.dram_tensor("x", [M, K], f32, kind="ExternalInput").ap()
    wt_ap = nc.dram_tensor("wt", [K, N_core], bf16, kind="ExternalInput").ap()
    sc_ap = nc.dram_tensor("sc", [1, N_core], f32, kind="ExternalInput").ap()
    bi_ap = nc.dram_tensor("bi", [1, N_core], f32, kind="ExternalInput").ap()
    out_ap = nc.dram_tensor("out", [M, N_core], f32, kind="ExternalOutput").ap()

    n_mchunks = M // m_chunk
    terms = 2 if split else 1

    with tile.TileContext(nc) as tc, ExitStack() as ctx:
        dram = ctx.enter_context(
            tc.tile_pool(name="dram", bufs=n_mchunks * terms, space="DRAM")
        )
        consts = ctx.enter_context(tc.tile_pool(name="consts", bufs=1))
        wt_pool = ctx.enter_context(tc.tile_pool(name="wt_pool", bufs=1))
        xt_pool = ctx.enter_context(tc.tile_pool(name="xt_pool", bufs=7))
        ev_pool = ctx.enter_context(tc.tile_pool(name="ev_pool", bufs=2))
        psum = ctx.enter_context(tc.tile_pool(name="psum", bufs=8, space="PSUM"))
        # XBAR transpose instructions block their issuing HWDGE engine for the
        # whole transfer; alternate between the two HWDGE engines (SP + ACT).
        hwdge = [nc.sync, nc.scalar]

        # Broadcast scale/bias across partitions once.
        sc_sb = consts.tile([P, N_core], f32, name="sc_sb")
        nc.scalar.dma_start(sc_sb[:], sc_ap.to_broadcast((P, N_core)))
        bi_sb = consts.tile([P, N_core], f32, name="bi_sb")
        nc.scalar.dma_start(bi_sb[:], bi_ap.to_broadcast((P, N_core)))

        # ---- Phase A: x f32 -> bf16 (and lo term when split) in DRAM ----
        xb_tiles = []  # per m-chunk, list of per-term DRAM tiles
        if not split:
            for mc in range(n_mchunks):
                xb = dram.tile([m_chunk, K], bf16, name=f"xb_{mc}", tag="xb")
                # Cast in P-row pieces: smaller transfers queue-block the
                # shared SDMA rings less than one monolithic chunk cast.
                for sub in range(m_chunk // P):
                    nc.gpsimd.dma_start(
                        out=xb[sub * P : (sub + 1) * P, :],
                        in_=x_ap[mc * m_chunk + sub * P : mc * m_chunk + (sub + 1) * P, :],
                    )
                xb_tiles.append([xb])
        else:
            a_pool = ctx.enter_context(tc.tile_pool(name="a_pool", bufs=3))
            ah_pool = ctx.enter_context(tc.tile_pool(name="ah_pool", bufs=3))
            al_pool = ctx.enter_context(tc.tile_pool(name="al_pool", bufs=3))
            a_chunk = min(A_CHUNK, K)
            for mc in range(n_mchunks):
                xbh = dram.tile([m_chunk, K], bf16, name=f"xbh_{mc}", tag="xbh")
                xbl = dram.tile([m_chunk, K], bf16, name=f"xbl_{mc}", tag="xbl")
                for sub in range(m_chunk // P):
                    r0 = mc * m_chunk + sub * P
                    for kc in range(K // a_chunk):
                        c0 = kc * a_chunk
                        a_in = a_pool.tile([P, a_chunk], f32, name="a_in")
                        nc.sync.dma_start(
                            a_in[:], x_ap[r0 : r0 + P, c0 : c0 + a_chunk]
                        )
                        a_hi = ah_pool.tile([P, a_chunk], bf16, name="a_hi")
                        nc.vector.tensor_copy(a_hi[:], a_in[:])
                        nc.sync.dma_start(
                            xbh[sub * P : sub * P + P, c0 : c0 + a_chunk], a_hi[:]
                        )
                        a_lo = al_pool.tile([P, a_chunk], bf16, name="a_lo")
                        nc.vector.tensor_sub(a_lo[:], a_in[:], a_hi[:])
                        nc.sync.dma_start(
                            xbl[sub * P : sub * P + P, c0 : c0 + a_chunk], a_lo[:]
                        )
                xb_tiles.append([xbh, xbl])

        # ---- Phase B: matmul sweep ----
        wt_re = wt_ap.rearrange("(ks p) n -> p ks n", p=P)
        for ncn in range(n_split):
            wt_sb = wt_pool.tile([P, KS, n_chunk], bf16, name=f"wt_{ncn}", tag="wt")
            nc.scalar.dma_start(
                wt_sb[:], wt_re[:, :, ncn * n_chunk : (ncn + 1) * n_chunk]
            )
            for mc in range(n_mchunks):
                ps = [
                    [
                        psum.tile([P, N_TILE], f32, name=f"ps_{msb}_{nt}", tag="ps")
                        for nt in range(nt_per)
                    ]
                    for msb in range(msb_per)
                ]
                for ti in range(terms):
                    xb = xb_tiles[mc][ti]
                    # Sub-tile the transposed x by ks-group so matmuls start
                    # after the first group lands instead of after all KS
                    # transposes, and slots recycle group-by-group.
                    ngrp = (KS + XT_G - 1) // XT_G
                    xts = []
                    for g in range(ngrp):
                        gsz = min(XT_G, KS - g * XT_G)
                        xt = xt_pool.tile(
                            [P, XT_G, m_chunk], bf16, name="xt", tag="xt"
                        )
                        xts.append(xt)
                        for kg in range(gsz):
                            ks = g * XT_G + kg
                            nc.sync.dma_start_transpose(
                                xt[:, kg, :], xb[:, ks * P : (ks + 1) * P]
                            )
                    for ks in range(KS):
                        g, kg = divmod(ks, XT_G)
                        for msb in range(msb_per):
                            lhsT = xts[g][:, kg, msb * P : (msb + 1) * P]
                            for nt in range(nt_per):
                                nc.tensor.matmul(
                                    ps[msb][nt][:],
                                    lhsT,
                                    wt_sb[:, ks, nt * N_TILE : (nt + 1) * N_TILE],
                                    start=(ti == 0 and ks == 0),
                                    stop=(ti == terms - 1 and ks == KS - 1),
                                )
                for msb in range(msb_per):
                    r0 = mc * m_chunk + msb * P
                    for nt in range(nt_per):
                        c0 = ncn * n_chunk + nt * N_TILE
                        ev = ev_pool.tile([P, N_TILE], f32, name="ev", tag="ev")
                        nc.vector.tensor_mul(
                            ev[:], ps[msb][nt][:], sc_sb[:, c0 : c0 + N_TILE]
                        )
                        nc.vector.tensor_add(
                            ev[:], ev[:], bi_sb[:, c0 : c0 + N_TILE]
                        )
                        nc.scalar.dma_start(
                            out_ap[r0 : r0 + P, c0 : c0 + N_TILE], ev[:]
                        )

    nc.compile()
    return nc


_BUILT = {}


def _get_module():
    key = (M_FULL, K_FULL, N_CORE_FULL, M_CHUNK, N_SPLIT, SPLIT)
    if key not in _BUILT:
        _BUILT[key] = build_module(
            M_FULL, K_FULL, N_CORE_FULL, M_CHUNK, N_SPLIT, SPLIT
        )
    return _BUILT[key]


def kernel(x, qweight, scales, bias):
    bf = ml_dtypes.bfloat16
    x2d = np.ascontiguousarray(x.reshape(M_FULL, K_FULL).astype(np.float32, copy=False))
    scales = np.asarray(scales, dtype=np.float32).reshape(DOUT)
    bias = np.asarray(bias, dtype=np.float32).reshape(DOUT)

    in_maps = []
    for c in range(N_CORES):
        lo, hi = c * N_CORE_FULL, (c + 1) * N_CORE_FULL
        # Weight repack: transpose to [K, Nc]; int values <= 15 are exact in bf16.
        wt_c = np.ascontiguousarray(qweight[lo:hi, :].T).astype(bf)
        in_maps.append(
            {
                "x": x2d,
                "wt": wt_c,
                "sc": scales[lo:hi].reshape(1, N_CORE_FULL),
                "bi": bias[lo:hi].reshape(1, N_CORE_FULL),
            }
        )

    nc = _get_module()
    trace = os.environ.get("AWQ_TRACE", "0") == "1"
    res = run_bass_kernel_spmd(
        nc, in_maps, core_ids=list(range(N_CORES)), trace=trace
    )
    if trace:
        kernel.last_exec_time_ns = res.exec_time_ns
        kernel.last_results = res

    out = np.empty((M_FULL, DOUT), dtype=np.float32)
    for c in range(N_CORES):
        out[:, c * N_CORE_FULL : (c + 1) * N_CORE_FULL] = res.results[c]["out"]
    return out.reshape(B, S, DOUT)


# revision 13
# speedup vs baseline: 1.7998x; 1.3786x over previous
"""AWQ linear kernel for Trainium2, 8-core column-parallel.

Computes y = x @ (qweight * scales).T + bias with
  x: [4, 4096, 4096] f32, qweight: [16384, 4096] int32 (values in [-15, 15]),
  scales: [16384, 1] f32, bias: [16384] f32.

Sharding: qweight/scales/bias are split along out_features across 8 cores
(column-parallel); x is replicated; each core computes its [M, 2048] output
shard and the host concatenates.

Math strategy: the integer qweight values are exactly representable in bf16,
so the matmul runs in bf16 against the *raw* integer weights and the
per-output-channel scale is applied to the fp32 PSUM result afterwards
(mathematically identical to dequantize-then-matmul). The only quantization
error is the bf16 rounding of x. An optional SPLIT mode represents
x = hi + lo with two bf16 arrays and accumulates both matmuls into the same
PSUM for near-fp32 accuracy at 2x PE cost.

Device-side data flow (per core):
  phase A: x f32 [M, K] -> bf16 [M, K] DRAM scratch (gpsimd cast-DMA, or
           DVE cast when SPLIT needs the hi/lo subtraction)
  phase B: XBAR DMA-transpose loads of x_bf16 -> SBUF [k, m] tiles;
           weights (host-pretransposed [K, Nc] bf16) resident in SBUF;
           PE matmuls accumulate over K into PSUM; DVE applies
           scale/bias on eviction; result DMA'd to DRAM.
"""

import os
from contextlib import ExitStack

import numpy as np
import ml_dtypes

import concourse.bass as bass
import concourse.tile as tile
from concourse import bacc, mybir
from concourse.bass_utils import run_bass_kernel_spmd

P = 128

# Full-problem constants
B, S, DIN, DOUT = 4, 4096, 4096, 16384
M_FULL = B * S          # 16384 rows of x
K_FULL = DIN            # 4096 contraction
N_CORES = 8
N_CORE_FULL = DOUT // N_CORES  # 2048 output features per core

# Tunables
M_CHUNK = int(os.environ.get("AWQ_M_CHUNK", "256"))   # x rows per compute chunk
N_SPLIT = int(os.environ.get("AWQ_N_SPLIT", "1"))     # weight residency chunks
SPLIT = os.environ.get("AWQ_SPLIT", "0") == "1"       # hi/lo x split (accuracy)
N_TILE = 512                                          # matmul moving free dim
A_CHUNK = 2048                                        # phase-A DVE chunk cols
XT_G = int(os.environ.get("AWQ_XT_G", "8"))           # ks per xt sub-tile group


def build_module(M, K, N_core, m_chunk, n_split, split, enable_asserts=False):
    """Emit the full tile program for one core (same program for all cores)."""
    KS = K // P
    assert M % m_chunk == 0 and m_chunk % P == 0
    assert N_core % n_split == 0
    n_chunk = N_core // n_split
    assert n_chunk % N_TILE == 0
    nt_per = n_chunk // N_TILE
    msb_per = m_chunk // P
    f32 = mybir.dt.float32
    bf16 = mybir.dt.bfloat16

    nc = bacc.Bacc(
        "TRN2",
        target_bir_lowering=False,
        debug=False,
        enable_asserts=enable_asserts,
        num_devices=N_CORES,
    )

    x_ap = nc.dram_tensor("x", [M, K], f32, kind="ExternalInput").ap()
    wt_ap = nc.dram_tensor("wt", [K, N_core], bf16, kind="ExternalInput").ap()
    sc_ap = nc.dram_tensor("sc", [1, N_core], f32, kind="ExternalInput").ap()
    bi_ap = nc.dram_tensor("bi", [1, N_core], f32, kind="ExternalInput").ap()
    out_ap = nc.dram_tensor("out", [M, N_core], f32, kind="ExternalOutput").ap()

    n_mchunks = M // m_chunk
    terms = 2 if split else 1

    with tile.TileContext(nc) as tc, ExitStack() as ctx:
        dram = ctx.enter_context(
            tc.tile_pool(name="dram", bufs=n_mchunks * terms, space="DRAM")
        )
        consts = ctx.enter_context(tc.tile_pool(name="consts", bufs=1))
        wt_pool = ctx.enter_context(tc.tile_pool(name="wt_pool", bufs=1))
        xt_pool = ctx.enter_context(tc.tile_pool(name="xt_pool", bufs=7))
        ev_pool = ctx.enter_context(tc.tile_pool(name="ev_pool", bufs=2))
        psum = ctx.enter_context(tc.tile_pool(name="psum", bufs=8, space="PSUM"))
        # XBAR transpose instructions block their issuing HWDGE engine for the
        # whole transfer; alternate between the two HWDGE engines (SP + ACT).
        hwdge = [nc.sync, nc.scalar]

        # Broadcast scale/bias across partitions once.
        sc_sb = consts.tile([P, N_core], f32, name="sc_sb")
        nc.scalar.dma_start(sc_sb[:], sc_ap.to_broadcast((P, N_core)))
        bi_sb = consts.tile([P, N_core], f32, name="bi_sb")
        nc.scalar.dma_start(bi_sb[:], bi_ap.to_broadcast((P, N_core)))

        # ---- Phase A: x f32 -> bf16 (and lo term when split) in DRAM ----
        xb_tiles = []  # per m-chunk, list of per-term DRAM tiles
        if not split:
            for mc in range(n_mchunks):
                xb = dram.tile([m_chunk, K], bf16, name=f"xb_{mc}", tag="xb")
                nc.gpsimd.dma_start(
                    out=xb[:], in_=x_ap[mc * m_chunk : (mc + 1) * m_chunk, :]
                )
                xb_tiles.append([xb])
        else:
            a_pool = ctx.enter_context(tc.tile_pool(name="a_pool", bufs=3))
            ah_pool = ctx.enter_context(tc.tile_pool(name="ah_pool", bufs=3))
            al_pool = ctx.enter_context(tc.tile_pool(name="al_pool", bufs=3))
            a_chunk = min(A_CHUNK, K)
            for mc in range(n_mchunks):
                xbh = dram.tile([m_chunk, K], bf16, name=f"xbh_{mc}", tag="xbh")
                xbl = dram.tile([m_chunk, K], bf16, name=f"xbl_{mc}", tag="xbl")
                for sub in range(m_chunk // P):
                    r0 = mc * m_chunk + sub * P
                    for kc in range(K // a_chunk):
                        c0 = kc * a_chunk
                        a_in = a_pool.tile([P, a_chunk], f32, name="a_in")
                        nc.sync.dma_start(
                            a_in[:], x_ap[r0 : r0 + P, c0 : c0 + a_chunk]
                        )
                        a_hi = ah_pool.tile([P, a_chunk], bf16, name="a_hi")
                        nc.vector.tensor_copy(a_hi[:], a_in[:])
                        nc.sync.dma_start(
                            xbh[sub * P : sub * P + P, c0 : c0 + a_chunk], a_hi[:]
                        )
                        a_lo = al_pool.tile([P, a_chunk], bf16, name="a_lo")
                        nc.vector.tensor_sub(a_lo[:], a_in[:], a_hi[:])
                        nc.sync.dma_start(
                            xbl[sub * P : sub * P + P, c0 : c0 + a_chunk], a_lo[:]
                        )
                xb_tiles.append([xbh, xbl])

        # ---- Phase B: matmul sweep ----
        wt_re = wt_ap.rearrange("(ks p) n -> p ks n", p=P)
        for ncn in range(n_split):
            wt_sb = wt_pool.tile([P, KS, n_chunk], bf16, name=f"wt_{ncn}", tag="wt")
            nc.scalar.dma_start(
                wt_sb[:], wt_re[:, :, ncn * n_chunk : (ncn + 1) * n_chunk]
            )
            for mc in range(n_mchunks):
                ps = [
                    [
                        psum.tile([P, N_TILE], f32, name=f"ps_{msb}_{nt}", tag="ps")
                        for nt in range(nt_per)
                    ]
                    for msb in range(msb_per)
                ]
                for ti in range(terms):
                    xb = xb_tiles[mc][ti]
                    # Sub-tile the transposed x by ks-group so matmuls start
                    # after the first group lands instead of after all KS
                    # transposes, and slots recycle group-by-group. One
                    # DMA_TRANSPOSE per group (3-D dst transposes XT_G
                    # k-subtiles at once) keeps the DMA instruction count low
                    # enough that Tile's in-flight-DMA window spans chunks.
                    ngrp = (KS + XT_G - 1) // XT_G
                    xts = []
                    for g in range(ngrp):
                        gsz = min(XT_G, KS - g * XT_G)
                        xt = xt_pool.tile(
                            [P, XT_G, m_chunk], bf16, name="xt", tag="xt"
                        )
                        xts.append(xt)
                        nc.sync.dma_start_transpose(
                            xt[:, :gsz, :],
                            xb[:, g * XT_G * P : (g * XT_G + gsz) * P],
                        )
                    for ks in range(KS):
                        g, kg = divmod(ks, XT_G)
                        for msb in range(msb_per):
                            lhsT = xts[g][:, kg, msb * P : (msb + 1) * P]
                            for nt in range(nt_per):
                                nc.tensor.matmul(
                                    ps[msb][nt][:],
                                    lhsT,
                                    wt_sb[:, ks, nt * N_TILE : (nt + 1) * N_TILE],
                                    start=(ti == 0 and ks == 0),
                                    stop=(ti == terms - 1 and ks == KS - 1),
                                )
                for msb in range(msb_per):
                    r0 = mc * m_chunk + msb * P
                    ev = ev_pool.tile([P, nt_per, N_TILE], f32, name="ev", tag="ev")
                    for nt in range(nt_per):
                        c0 = ncn * n_chunk + nt * N_TILE
                        nc.vector.tensor_mul(
                            ev[:, nt, :], ps[msb][nt][:], sc_sb[:, c0 : c0 + N_TILE]
                        )
                        nc.vector.tensor_add(
                            ev[:, nt, :], ev[:, nt, :], bi_sb[:, c0 : c0 + N_TILE]
                        )
                    nc.scalar.dma_start(
                        out_ap[r0 : r0 + P, ncn * n_chunk : (ncn + 1) * n_chunk],
                        ev[:, :, :],
                    )

    nc.compile()
    return nc


_BUILT = {}


def _get_module():
    key = (M_FULL, K_FULL, N_CORE_FULL, M_CHUNK, N_SPLIT, SPLIT)
    if key not in _BUILT:
        _BUILT[key] = build_module(
            M_FULL, K_FULL, N_CORE_FULL, M_CHUNK, N_SPLIT, SPLIT
        )
    return _BUILT[key]


def kernel(x, qweight, scales, bias):
    bf = ml_dtypes.bfloat16
    x2d = np.ascontiguousarray(x.reshape(M_FULL, K_FULL).astype(np.float32, copy=False))
    scales = np.asarray(scales, dtype=np.float32).reshape(DOUT)
    bias = np.asarray(bias, dtype=np.float32).reshape(DOUT)

    in_maps = []
    for c in range(N_CORES):
        lo, hi = c * N_CORE_FULL, (c + 1) * N_CORE_FULL
        # Weight repack: transpose to [K, Nc]; int values <= 15 are exact in bf16.
        wt_c = np.ascontiguousarray(qweight[lo:hi, :].T).astype(bf)
        in_maps.append(
            {
                "x": x2d,
                "wt": wt_c,
                "sc": scales[lo:hi].reshape(1, N_CORE_FULL),
                "bi": bias[lo:hi].reshape(1, N_CORE_FULL),
            }
        )

    nc = _get_module()
    trace = os.environ.get("AWQ_TRACE", "0") == "1"
    res = run_bass_kernel_spmd(
        nc, in_maps, core_ids=list(range(N_CORES)), trace=trace
    )
    if trace:
        kernel.last_exec_time_ns = res.exec_time_ns
        kernel.last_results = res

    out = np.empty((M_FULL, DOUT), dtype=np.float32)
    for c in range(N_CORES):
        out[:, c * N_CORE_FULL : (c + 1) * N_CORE_FULL] = res.results[c]["out"]
    return out.reshape(B, S, DOUT)
